# revision 33
# baseline (speedup 1.0000x reference)
"""Multi-head self-attention (B=4, N=2048, C=512, H=8) on 8 trn2 NeuronCores.

Sharding: core = 2*b + g (b = batch, g = head-half). Each core handles one
batch element and 4 heads (2 head-pairs j); host sums the two partial
projections per batch element and adds b_proj.

v2 design (all attention matmuls bf16, fp32 PSUM accumulation):
  1. qkv: q^T/k^T per pair j as [128, 2048] bf16 (head 2j on partitions
     0-63, head 2j+1 on 64-127); bias folded in via ACT Identity-with-bias
     copies. v packed per key-tile as [128, 4, 66] bf16 with a ones column
     at index 64 (softmax-denominator trick).
  2. attention per (j, q-chunk of 512, key-tile m): the two heads' score
     matmuls use disjoint PE row quadrants (tile_position (0,0)/(64,0)) so
     they stream concurrently. exp is split between ACT (exact, bf16 out)
     and DVE (Schraudolph bits: round(s*A+B) -> uint16, bitcast bf16).
     out^T[65] accumulates v|1 against p; row 64 = denominator.
  3. normalize: ACT Reciprocal on the denominator row (fused PSUM read),
     DVE multiply with a partition-broadcast AP.
  4. projection: y^T accumulated over the two pairs, fp32 out.
"""

import numpy as np

import concourse.bacc as bacc
import concourse.bass as bass
import concourse.mybir as mybir
import concourse.tile as tile
from concourse.bass_utils import run_bass_kernel_spmd

B, N, C, H, HD = 4, 2048, 512, 8, 64
HPC, CS = 4, 256  # heads per core, channels per core
SCALE = HD ** -0.5
F32 = mybir.dt.float32
F32R = mybir.dt.float32r
BF16 = mybir.dt.bfloat16
U16 = mybir.dt.uint16
NCORES = 8
MT = N // 128  # 16 key tiles

LOG2E = float(np.log2(np.e))
SCH_A = SCALE * 128.0 * LOG2E   # schraudolph scale (bf16 bits)
SCH_B = 16256.0 - 5.5           # 127<<7 minus minimax correction

# which key-tiles m use the DVE schraudolph exp (rest use exact ACT exp)
DVE_MS = frozenset({1, 4, 7, 10, 13})

_NC = None


def _build(reps=1, dump=False, do_qkv=True, do_attn=True, do_proj=True,
           attn_mode="full"):
    nc = bacc.Bacc("TRN2", target_bir_lowering=False, debug=False,
                   num_devices=NCORES)
    if dump:
        qT_dump = [nc.dram_tensor(f"qT{j}_dump", [128, N], BF16,
                                  kind="ExternalOutput") for j in range(2)]
        kT_dump = [nc.dram_tensor(f"kT{j}_dump", [128, N], BF16,
                                  kind="ExternalOutput") for j in range(2)]
        v_dump = nc.dram_tensor("v_dump", [128, HPC * (HD + 2)], BF16,
                                kind="ExternalOutput")
        o_dump = [nc.dram_tensor(f"o{j}_dump", [128, N], BF16,
                                 kind="ExternalOutput") for j in range(2)]
        st_dump = nc.dram_tensor("st_dump", [128, 2048], F32,
                                 kind="ExternalOutput")
        p_dump = nc.dram_tensor("p_dump", [128, 2048], BF16,
                                kind="ExternalOutput")
        ot_dump = nc.dram_tensor("ot_dump", [HD + 1, 1024], F32,
                                 kind="ExternalOutput")
        rc_dump = nc.dram_tensor("rc_dump", [1, 1024], F32,
                                 kind="ExternalOutput")
    xT_d = nc.dram_tensor("xT", [C, N], BF16, kind="ExternalInput")
    wqT_d = nc.dram_tensor("wqT", [C, CS], BF16, kind="ExternalInput")
    wkT_d = nc.dram_tensor("wkT", [C, CS], BF16, kind="ExternalInput")
    wvT_d = nc.dram_tensor("wvT", [C, CS], BF16, kind="ExternalInput")
    wpT_d = nc.dram_tensor("wpT", [CS, C], BF16, kind="ExternalInput")
    bq_d = nc.dram_tensor("bq", [128, 2], F32, kind="ExternalInput")
    bk_d = nc.dram_tensor("bk", [128, 2], F32, kind="ExternalInput")
    bv_d = nc.dram_tensor("bv", [1, CS], BF16, kind="ExternalInput")
    ones4_d = nc.dram_tensor("ones4", [128, HPC], BF16, kind="ExternalInput")
    ones_row_d = nc.dram_tensor("ones_row", [1, 128], BF16,
                                kind="ExternalInput")
    yT_d = nc.dram_tensor("yT", [C, N], F32, kind="ExternalOutput")

    with tile.TileContext(nc) as tc:
      def body():
          with (
              tc.tile_pool(name="const", bufs=1) as const,
              tc.tile_pool(name="big", bufs=1) as big,
              tc.tile_pool(name="pexp", bufs=4) as pexp,
              tc.tile_pool(name="psch", bufs=4) as psch,
              tc.tile_pool(name="rc", bufs=2) as rcp,
              tc.tile_pool(name="rbc", bufs=2) as rbcp,
              tc.tile_pool(name="ysb", bufs=3) as ysbp,
          ):
              # ---- input DMA ------------------------------------------------
              xt = [big.tile([128, N], BF16, tag=f"x{ct}", name=f"x{ct}")
                    for ct in range(4)]
              wq_t, wk_t, wv_t = [], [], []
              for ct in range(4):
                  for lst, nm in ((wq_t, "wq"), (wk_t, "wk"), (wv_t, "wv")):
                      lst.append(const.tile([128, CS], BF16, tag=f"{nm}{ct}",
                                            name=f"{nm}{ct}"))
              for ct in range(4):
                  nc.sync.dma_start(out=xt[ct][:],
                                    in_=xT_d[bass.ts(ct, 128), :])
                  nc.gpsimd.dma_start(out=wk_t[ct][:],
                                      in_=wkT_d[bass.ts(ct, 128), :])
                  nc.gpsimd.dma_start(out=wq_t[ct][:],
                                      in_=wqT_d[bass.ts(ct, 128), :])
                  nc.gpsimd.dma_start(out=wv_t[ct][:],
                                      in_=wvT_d[bass.ts(ct, 128), :])
              bq_sb = const.tile([128, 2], F32, tag="bq", name="bq")
              nc.gpsimd.dma_start(out=bq_sb[:], in_=bq_d[:])
              bk_sb = const.tile([128, 2], F32, tag="bk", name="bk")
              nc.gpsimd.dma_start(out=bk_sb[:], in_=bk_d[:])
              bv_sb = const.tile([1, CS], BF16, tag="bv", name="bv")
              nc.gpsimd.dma_start(out=bv_sb[:], in_=bv_d[:])
              ones_row = const.tile([1, 128], BF16, tag="ones_row",
                                    name="ones_row")
              nc.gpsimd.dma_start(out=ones_row[:], in_=ones_row_d[:])
              wp_t = []
              for j in range(2):
                  t = const.tile([128, C], BF16, tag=f"wp{j}", name=f"wp{j}")
                  nc.gpsimd.dma_start(out=t[:], in_=wpT_d[bass.ts(j, 128), :])
                  wp_t.append(t)

              # ---- persistent activations -----------------------------------
              qT = [big.tile([128, N], BF16, tag=f"qT{j}", name=f"qT{j}")
                    for j in range(2)]
              kT = [big.tile([128, N], BF16, tag=f"kT{j}", name=f"kT{j}")
                    for j in range(2)]
              v1m = [big.tile([128, HPC, HD + 2], BF16, tag=f"v1m_{m}",
                              name=f"v1m_{m}") for m in range(MT)]
              for m in range(MT):
                  nc.gpsimd.dma_start(
                      out=v1m[m][:, :, HD:HD + 1],
                      in_=ones4_d[:, :].rearrange("p (h o) -> p h o", o=1),
                  )
              oT_sb = [big.tile([128, N], BF16, tag=f"oT{j}", name=f"oT{j}")
                       for j in range(2)]

              if not do_qkv:
                  for j in range(2):
                      nc.sync.dma_start(out=qT[j][:],
                                        in_=xT_d[bass.ts(j, 128), :])
                      nc.sync.dma_start(out=kT[j][:],
                                        in_=xT_d[bass.ts(2 + j, 128), :])
                  for m in range(MT):
                      nc.gpsimd.dma_start(
                          out=v1m[m][:, :, 0:HD],
                          in_=xT_d[0:128, m * 64:m * 64 + 256]
                          .rearrange("p (h d) -> p h d", h=HPC))

              # ---- phase 1: qkv ---------------------------------------------
              with (
                  tc.tile_pool(name="qkps", bufs=2, space="PSUM") as qkps,
                  tc.tile_pool(name="vps", bufs=2, space="PSUM") as vps,
              ):
                  def qk_pair(j):
                      for w_t, b_sb, dst in ((wk_t, bk_sb, kT),
                                             (wq_t, bq_sb, qT)):
                          for nkq in range(4):
                              ps = qkps.tile([128, 512], F32, tag="qk",
                                             name="qk")
                              for ct in range(4):
                                  nc.tensor.matmul(
                                      ps[:],
                                      lhsT=w_t[ct][:, bass.ts(j, 128)],
                                      rhs=xt[ct][:, bass.ts(nkq, 512)],
                                      start=(ct == 0), stop=(ct == 3),
                                  )
                              nc.vector.tensor_scalar_add(
                                  dst[j][:, bass.ts(nkq, 512)], ps[:],
                                  b_sb[:, j:j + 1])

                  if do_qkv:
                      qk_pair(0)
                      for m in range(MT):
                          vp = vps.tile([128, CS], F32, tag="v", name="v")
                          for ct in range(4):
                              nc.tensor.matmul(
                                  vp[:],
                                  lhsT=xt[ct][:, bass.ts(m, 128)],
                                  rhs=wv_t[ct][:],
                                  start=(ct == 0), stop=False,
                              )
                          nc.tensor.matmul(vp[:], lhsT=ones_row[:],
                                           rhs=bv_sb[:],
                                           start=False, stop=True)
                          nc.vector.tensor_copy(v1m[m][:, :, 0:HD], vp[:])
                      qk_pair(1)

              # ---- phase 2: attention ---------------------------------------
              pfix = None
              if attn_mode == "noexp":
                  pfix = big.tile([128, 512], BF16, tag="pfix", name="pfix")
                  nc.sync.dma_start(out=pfix[:], in_=xT_d[0:128, 0:512])
              with (
                  tc.tile_pool(name="stps", bufs=2, space="PSUM") as stps,
                  tc.tile_pool(name="otps", bufs=2, space="PSUM") as otps,
              ):
                  for j in range(2):
                      for q5 in range(4):
                          qsl = bass.ts(q5, 512)
                          oTa = otps.tile([HD + 1, 512], F32, tag="oa",
                                          name="oa")
                          oTb = otps.tile([HD + 1, 512], F32, tag="ob",
                                          name="ob")
                          for m in range(MT):
                              sT = stps.tile([128, 1024], F32, tag="s",
                                             name="s")
                              sTa, sTb = sT[:, 0:512], sT[:, 512:1024]
                              nc.tensor.matmul(
                                  sTa, lhsT=kT[j][0:64, bass.ts(m, 128)],
                                  rhs=qT[j][0:64, qsl],
                                  start=True, stop=True)
                              nc.tensor.matmul(
                                  sTb, lhsT=kT[j][64:128, bass.ts(m, 128)],
                                  rhs=qT[j][64:128, qsl],
                                  start=True, stop=True)
                              rhs_ab = []
                              if attn_mode == "noexp":
                                  rhs_ab = [pfix[:], pfix[:]]
                                  if m == MT - 1:
                                      ds = rcp.tile([128, 1024], F32,
                                                    tag="ds", name="ds")
                                      nc.vector.tensor_copy(ds[:], sT[:])
                              elif m in DVE_MS:
                                  pt = psch.tile([128, 1024], U16,
                                                 tag="ps", name="ps")
                                  nc.vector.tensor_scalar(
                                      out=pt[:], in0=sT[:],
                                      scalar1=SCH_A, scalar2=SCH_B,
                                      op0=mybir.AluOpType.mult,
                                      op1=mybir.AluOpType.add)
                                  rhs_ab = [pt[:, 0:512].bitcast(BF16),
                                            pt[:, 512:1024].bitcast(BF16)]
                              else:
                                  pt = pexp.tile([128, 1024], BF16,
                                                 tag="pe", name="pe")
                                  nc.scalar.activation(
                                      out=pt[:], in_=sT[:],
                                      func=mybir.ActivationFunctionType.Exp,
                                      scale=SCALE)
                                  rhs_ab = [pt[:, 0:512], pt[:, 512:1024]]
                              if dump and j == 0 and q5 == 0 and m in (0, 1):
                                  dsb = rcp.tile([128, 1024], F32, tag="dsb",
                                                 name="dsb")
                                  nc.vector.tensor_copy(dsb[:], sT[:])
                                  nc.sync.dma_start(
                                      out=st_dump[:, bass.ts(m, 1024)],
                                      in_=dsb[:])
                                  nc.sync.dma_start(
                                      out=p_dump[:, m * 1024:m * 1024 + 512],
                                      in_=rhs_ab[0])
                                  nc.sync.dma_start(
                                      out=p_dump[:, m * 1024 + 512:
                                                  (m + 1) * 1024],
                                      in_=rhs_ab[1])
                              for hh, (oT, rhs) in enumerate(
                                      zip((oTa, oTb), rhs_ab)):
                                  nc.tensor.matmul(
                                      oT[:],
                                      lhsT=v1m[m][:, 2 * j + hh, 0:HD + 1],
                                      rhs=rhs,
                                      start=(m == 0), stop=(m == MT - 1))
                          for hh, oT in enumerate((oTa, oTb)):
                              den = rcp.tile([1, 512], F32, tag="den",
                                             name="den")
                              nc.vector.tensor_copy(den[:], oT[HD:HD + 1, :])
                              rc = rcp.tile([1, 512], F32, tag="rc", name="rc")
                              nc.vector.reciprocal_approx_fast(
                                  out=rc[:], in_=den[:])
                              if dump and j == 0 and q5 == 0:
                                  dot = rcp.tile([HD + 1, 512], F32,
                                                 tag="dot", name="dot")
                                  nc.vector.tensor_copy(dot[:], oT[:])
                                  nc.sync.dma_start(
                                      out=ot_dump[:, hh * 512:(hh + 1) * 512],
                                      in_=dot[:])
                                  nc.sync.dma_start(
                                      out=rc_dump[:, hh * 512:(hh + 1) * 512],
                                      in_=rc[:])
                              bc = rbcp.tile([HD, 512], F32, tag="bc",
                                             name="bc")
                              nc.gpsimd.partition_broadcast(bc[:], rc[:])
                              nc.vector.tensor_mul(
                                  oT_sb[j][bass.ts(hh, 64), qsl],
                                  oT[0:HD, :], bc[:])

              if dump:
                  for j in range(2):
                      nc.sync.dma_start(out=qT_dump[j][:], in_=qT[j][:])
                      nc.sync.dma_start(out=kT_dump[j][:], in_=kT[j][:])
                      nc.sync.dma_start(out=o_dump[j][:], in_=oT_sb[j][:])
                  nc.sync.dma_start(
                      out=v_dump[:],
                      in_=v1m[0][:].rearrange("p h d -> p (h d)"))

              # ---- phase 3: projection --------------------------------------
              if not do_proj:
                  for j in range(2):
                      nc.scalar.dma_start(
                          out=yT_d[bass.ts(j, 128), :].bitcast(BF16)[:, 0:N],
                          in_=oT_sb[j][:])
                  return
              with tc.tile_pool(name="yps", bufs=4, space="PSUM") as yps:
                  for jj in range(4):
                      for tch in range(4):
                          yp = yps.tile([128, 512], F32, tag="yp", name="yp")
                          for j in range(2):
                              nc.tensor.matmul(
                                  yp[:],
                                  lhsT=wp_t[j][:, bass.ts(jj, 128)],
                                  rhs=oT_sb[j][:, bass.ts(tch, 512)],
                                  start=(j == 0), stop=(j == 1))
                          ys = ysbp.tile([128, 512], F32, tag="ys", name="ys")
                          if jj % 2 == 0:
                              nc.scalar.copy(ys[:], yp[:])
                          else:
                              nc.vector.tensor_copy(ys[:], yp[:])
                          nc.scalar.dma_start(
                              out=yT_d[bass.ts(jj, 128), bass.ts(tch, 512)],
                              in_=ys[:])

      if reps > 1:
          with tc.For_i(0, reps, 1):
              body()
      else:
          body()

    nc.compile()
    return nc


def get_nc():
    global _NC
    if _NC is None:
        _NC = _build()
    return _NC


def build_timing_nc(reps):
    return _build(reps=reps)


def shard_inputs(x, w_qkv, b_qkv, w_proj, b_proj):
    import ml_dtypes

    bf16 = ml_dtypes.bfloat16
    x = np.asarray(x, dtype=np.float32)
    w_qkv = np.asarray(w_qkv, dtype=np.float32)
    b_qkv = np.asarray(b_qkv, dtype=np.float32)
    w_proj = np.asarray(w_proj, dtype=np.float32)
    ones4 = np.ones((128, HPC), bf16)
    ones_row = np.ones((1, 128), bf16)
    in_maps = []
    for core in range(NCORES):
        b, g = core // 2, core % 2
        sl = slice(g * CS, (g + 1) * CS)
        in_maps.append({
            "xT": np.ascontiguousarray(x[b].T).astype(bf16),
            "wqT": np.ascontiguousarray(w_qkv[sl, :].T).astype(bf16),
            "wkT": np.ascontiguousarray(w_qkv[C:][sl, :].T).astype(bf16),
            "wvT": np.ascontiguousarray(w_qkv[2 * C:][sl, :].T).astype(bf16),
            "wpT": np.ascontiguousarray(w_proj[:, sl].T).astype(bf16),
            "bq": np.ascontiguousarray(b_qkv[sl].reshape(2, 128).T),
            "bk": np.ascontiguousarray(b_qkv[C:][sl].reshape(2, 128).T),
            "bv": np.ascontiguousarray(
                b_qkv[2 * C:][sl].reshape(1, CS)).astype(bf16),
            "ones4": ones4,
            "ones_row": ones_row,
        })
    return in_maps


def gather_output(results, b_proj):
    b_proj = np.asarray(b_proj, dtype=np.float32)
    out = np.empty((B, N, C), np.float32)
    for b in range(B):
        yT = results[2 * b]["yT"] + results[2 * b + 1]["yT"]
        out[b] = yT.T + b_proj[None, :]
    return out


def kernel(x, w_qkv, b_qkv, w_proj, b_proj):
    nc = get_nc()
    in_maps = shard_inputs(x, w_qkv, b_qkv, w_proj, b_proj)
    res = run_bass_kernel_spmd(nc, in_maps, core_ids=list(range(NCORES)))
    return gather_output(res.results, b_proj)


# revision 36
# speedup vs baseline: 1.1046x; 1.1046x over previous
"""Multi-head self-attention (B=4, N=2048, C=512, H=8) on 8 trn2 NeuronCores.

Sharding: core = 2*b + g (b = batch, g = head-half). Each core handles one
batch element and 4 heads (2 head-pairs j); host sums the two partial
projections per batch element and adds b_proj.

v2 design (all attention matmuls bf16, fp32 PSUM accumulation):
  1. qkv: q^T/k^T per pair j as [128, 2048] bf16 (head 2j on partitions
     0-63, head 2j+1 on 64-127); bias folded in via ACT Identity-with-bias
     copies. v packed per key-tile as [128, 4, 66] bf16 with a ones column
     at index 64 (softmax-denominator trick).
  2. attention per (j, q-chunk of 512, key-tile m): the two heads' score
     matmuls use disjoint PE row quadrants (tile_position (0,0)/(64,0)) so
     they stream concurrently. exp is split between ACT (exact, bf16 out)
     and DVE (Schraudolph bits: round(s*A+B) -> uint16, bitcast bf16).
     out^T[65] accumulates v|1 against p; row 64 = denominator.
  3. normalize: ACT Reciprocal on the denominator row (fused PSUM read),
     DVE multiply with a partition-broadcast AP.
  4. projection: y^T accumulated over the two pairs, fp32 out.
"""

import numpy as np

import concourse.bacc as bacc
import concourse.bass as bass
import concourse.mybir as mybir
import concourse.tile as tile
from concourse.bass_utils import run_bass_kernel_spmd

B, N, C, H, HD = 4, 2048, 512, 8, 64
HPC, CS = 4, 256  # heads per core, channels per core
SCALE = HD ** -0.5
F32 = mybir.dt.float32
F32R = mybir.dt.float32r
BF16 = mybir.dt.bfloat16
U16 = mybir.dt.uint16
NCORES = 8
MT = N // 128  # 16 key tiles

LOG2E = float(np.log2(np.e))
SCH_A = SCALE * 128.0 * LOG2E   # schraudolph scale (bf16 bits)
SCH_B = 16256.0 - 5.5           # 127<<7 minus minimax correction

# which key-tiles m use the DVE schraudolph exp (rest use exact ACT exp)
DVE_MS = frozenset({2, 6, 10, 14})

_NC = None


def _build(reps=1, dump=False, do_qkv=True, do_attn=True, do_proj=True,
           attn_mode="full"):
    nc = bacc.Bacc("TRN2", target_bir_lowering=False, debug=False,
                   num_devices=NCORES)
    if dump:
        qT_dump = [nc.dram_tensor(f"qT{j}_dump", [128, N], BF16,
                                  kind="ExternalOutput") for j in range(2)]
        kT_dump = [nc.dram_tensor(f"kT{j}_dump", [128, N], BF16,
                                  kind="ExternalOutput") for j in range(2)]
        v_dump = nc.dram_tensor("v_dump", [128, HPC * (HD + 2)], BF16,
                                kind="ExternalOutput")
        o_dump = [nc.dram_tensor(f"o{j}_dump", [128, N], BF16,
                                 kind="ExternalOutput") for j in range(2)]
        st_dump = nc.dram_tensor("st_dump", [128, 2048], F32,
                                 kind="ExternalOutput")
        p_dump = nc.dram_tensor("p_dump", [128, 2048], BF16,
                                kind="ExternalOutput")
        ot_dump = nc.dram_tensor("ot_dump", [HD + 1, 1024], F32,
                                 kind="ExternalOutput")
        rc_dump = nc.dram_tensor("rc_dump", [1, 1024], F32,
                                 kind="ExternalOutput")
    xT_d = nc.dram_tensor("xT", [C, N], BF16, kind="ExternalInput")
    wqT_d = nc.dram_tensor("wqT", [C, CS], BF16, kind="ExternalInput")
    wkT_d = nc.dram_tensor("wkT", [C, CS], BF16, kind="ExternalInput")
    wvT_d = nc.dram_tensor("wvT", [C, CS], BF16, kind="ExternalInput")
    wpT_d = nc.dram_tensor("wpT", [CS, C], BF16, kind="ExternalInput")
    bq_d = nc.dram_tensor("bq", [128, 2], F32, kind="ExternalInput")
    bk_d = nc.dram_tensor("bk", [128, 2], F32, kind="ExternalInput")
    bv_d = nc.dram_tensor("bv", [1, CS], BF16, kind="ExternalInput")
    ones4_d = nc.dram_tensor("ones4", [128, HPC], BF16, kind="ExternalInput")
    ones_row_d = nc.dram_tensor("ones_row", [1, 128], BF16,
                                kind="ExternalInput")
    yT_d = nc.dram_tensor("yT", [C, N], F32, kind="ExternalOutput")

    with tile.TileContext(nc) as tc:
      def body():
          with (
              tc.tile_pool(name="const", bufs=1) as const,
              tc.tile_pool(name="big", bufs=1) as big,
              tc.tile_pool(name="pexp", bufs=4) as pexp,
              tc.tile_pool(name="psch", bufs=4) as psch,
              tc.tile_pool(name="rc", bufs=2) as rcp,
              tc.tile_pool(name="rbc", bufs=2) as rbcp,
              tc.tile_pool(name="ysb", bufs=3) as ysbp,
          ):
              # ---- input DMA ------------------------------------------------
              xt = [big.tile([128, N], BF16, tag=f"x{ct}", name=f"x{ct}")
                    for ct in range(4)]
              wq_t, wk_t, wv_t = [], [], []
              for ct in range(4):
                  for lst, nm in ((wq_t, "wq"), (wk_t, "wk"), (wv_t, "wv")):
                      lst.append(const.tile([128, CS], BF16, tag=f"{nm}{ct}",
                                            name=f"{nm}{ct}"))
              for ct in range(4):
                  nc.gpsimd.dma_start(out=wk_t[ct][:],
                                      in_=wkT_d[bass.ts(ct, 128), :])
                  nc.gpsimd.dma_start(out=wq_t[ct][:],
                                      in_=wqT_d[bass.ts(ct, 128), :])
                  nc.gpsimd.dma_start(out=wv_t[ct][:],
                                      in_=wvT_d[bass.ts(ct, 128), :])
              # x arrives in (token-chunk, ct) order on two queues so the
              # first qk accumulation group can start after ~1/4 of the x DMA
              for nkq in range(4):
                  for ct in range(4):
                      eng = nc.sync if ct % 2 == 0 else nc.scalar
                      eng.dma_start(
                          out=xt[ct][:, bass.ts(nkq, 512)],
                          in_=xT_d[bass.ts(ct, 128), bass.ts(nkq, 512)])
              bq_sb = const.tile([128, 2], F32, tag="bq", name="bq")
              nc.gpsimd.dma_start(out=bq_sb[:], in_=bq_d[:])
              bk_sb = const.tile([128, 2], F32, tag="bk", name="bk")
              nc.gpsimd.dma_start(out=bk_sb[:], in_=bk_d[:])
              bv_sb = const.tile([1, CS], BF16, tag="bv", name="bv")
              nc.gpsimd.dma_start(out=bv_sb[:], in_=bv_d[:])
              ones_row = const.tile([1, 128], BF16, tag="ones_row",
                                    name="ones_row")
              nc.gpsimd.dma_start(out=ones_row[:], in_=ones_row_d[:])
              wp_t = []
              for j in range(2):
                  t = const.tile([128, C], BF16, tag=f"wp{j}", name=f"wp{j}")
                  nc.gpsimd.dma_start(out=t[:], in_=wpT_d[bass.ts(j, 128), :])
                  wp_t.append(t)

              # ---- persistent activations -----------------------------------
              qT = [big.tile([128, N], BF16, tag=f"qT{j}", name=f"qT{j}")
                    for j in range(2)]
              kT = [big.tile([128, N], BF16, tag=f"kT{j}", name=f"kT{j}")
                    for j in range(2)]
              v1m = [big.tile([128, HPC, HD + 2], BF16, tag=f"v1m_{m}",
                              name=f"v1m_{m}") for m in range(MT)]
              for m in range(MT):
                  nc.gpsimd.dma_start(
                      out=v1m[m][:, :, HD:HD + 1],
                      in_=ones4_d[:, :].rearrange("p (h o) -> p h o", o=1),
                  )
              oT_sb = [big.tile([128, N], BF16, tag=f"oT{j}", name=f"oT{j}")
                       for j in range(2)]

              if not do_qkv:
                  for j in range(2):
                      nc.sync.dma_start(out=qT[j][:],
                                        in_=xT_d[bass.ts(j, 128), :])
                      nc.sync.dma_start(out=kT[j][:],
                                        in_=xT_d[bass.ts(2 + j, 128), :])
                  for m in range(MT):
                      nc.gpsimd.dma_start(
                          out=v1m[m][:, :, 0:HD],
                          in_=xT_d[0:128, m * 64:m * 64 + 256]
                          .rearrange("p (h d) -> p h d", h=HPC))

              # ---- phase 1: qkv ---------------------------------------------
              with (
                  tc.tile_pool(name="qkps", bufs=2, space="PSUM") as qkps,
                  tc.tile_pool(name="vps", bufs=2, space="PSUM") as vps,
              ):
                  def qk_pair(j):
                      for w_t, b_sb, dst in ((wk_t, bk_sb, kT),
                                             (wq_t, bq_sb, qT)):
                          for nkq in range(4):
                              ps = qkps.tile([128, 512], F32, tag="qk",
                                             name="qk")
                              for ct in range(4):
                                  nc.tensor.matmul(
                                      ps[:],
                                      lhsT=w_t[ct][:, bass.ts(j, 128)],
                                      rhs=xt[ct][:, bass.ts(nkq, 512)],
                                      start=(ct == 0), stop=(ct == 3),
                                  )
                              nc.vector.tensor_scalar_add(
                                  dst[j][:, bass.ts(nkq, 512)], ps[:],
                                  b_sb[:, j:j + 1])

                  if do_qkv:
                      qk_pair(0)
                      for m in range(MT):
                          vp = vps.tile([128, CS], F32, tag="v", name="v")
                          for ct in range(4):
                              nc.tensor.matmul(
                                  vp[:],
                                  lhsT=xt[ct][:, bass.ts(m, 128)],
                                  rhs=wv_t[ct][:],
                                  start=(ct == 0), stop=False,
                              )
                          nc.tensor.matmul(vp[:], lhsT=ones_row[:],
                                           rhs=bv_sb[:],
                                           start=False, stop=True)
                          nc.scalar.copy(v1m[m][:, :, 0:HD], vp[:])
                      qk_pair(1)

              # ---- phase 2: attention ---------------------------------------
              pfix = None
              if attn_mode == "noexp":
                  pfix = big.tile([128, 512], BF16, tag="pfix", name="pfix")
                  nc.sync.dma_start(out=pfix[:], in_=xT_d[0:128, 0:512])
              with (
                  tc.tile_pool(name="stps", bufs=2, space="PSUM") as stps,
                  tc.tile_pool(name="otps", bufs=2, space="PSUM") as otps,
              ):
                  for j in range(2):
                      for q5 in range(4):
                          qsl = bass.ts(q5, 512)
                          oTa = otps.tile([HD + 1, 512], F32, tag="oa",
                                          name="oa")
                          oTb = otps.tile([HD + 1, 512], F32, tag="ob",
                                          name="ob")
                          for m in range(MT):
                              sT = stps.tile([128, 1024], F32, tag="s",
                                             name="s")
                              sTa, sTb = sT[:, 0:512], sT[:, 512:1024]
                              nc.tensor.matmul(
                                  sTa, lhsT=kT[j][0:64, bass.ts(m, 128)],
                                  rhs=qT[j][0:64, qsl],
                                  start=True, stop=True)
                              nc.tensor.matmul(
                                  sTb, lhsT=kT[j][64:128, bass.ts(m, 128)],
                                  rhs=qT[j][64:128, qsl],
                                  start=True, stop=True)
                              rhs_ab = []
                              if attn_mode == "noexp":
                                  rhs_ab = [pfix[:], pfix[:]]
                                  if m == MT - 1:
                                      ds = rcp.tile([128, 1024], F32,
                                                    tag="ds", name="ds")
                                      nc.vector.tensor_copy(ds[:], sT[:])
                              elif m in DVE_MS:
                                  pt = psch.tile([128, 1024], U16,
                                                 tag="ps", name="ps")
                                  nc.vector.tensor_scalar(
                                      out=pt[:], in0=sT[:],
                                      scalar1=SCH_A, scalar2=SCH_B,
                                      op0=mybir.AluOpType.mult,
                                      op1=mybir.AluOpType.add)
                                  rhs_ab = [pt[:, 0:512].bitcast(BF16),
                                            pt[:, 512:1024].bitcast(BF16)]
                              else:
                                  pt = pexp.tile([128, 1024], BF16,
                                                 tag="pe", name="pe")
                                  nc.scalar.activation(
                                      out=pt[:], in_=sT[:],
                                      func=mybir.ActivationFunctionType.Exp,
                                      scale=SCALE)
                                  rhs_ab = [pt[:, 0:512], pt[:, 512:1024]]
                              if dump and j == 0 and q5 == 0 and m in (0, 1):
                                  dsb = rcp.tile([128, 1024], F32, tag="dsb",
                                                 name="dsb")
                                  nc.vector.tensor_copy(dsb[:], sT[:])
                                  nc.sync.dma_start(
                                      out=st_dump[:, bass.ts(m, 1024)],
                                      in_=dsb[:])
                                  nc.sync.dma_start(
                                      out=p_dump[:, m * 1024:m * 1024 + 512],
                                      in_=rhs_ab[0])
                                  nc.sync.dma_start(
                                      out=p_dump[:, m * 1024 + 512:
                                                  (m + 1) * 1024],
                                      in_=rhs_ab[1])
                              for hh, (oT, rhs) in enumerate(
                                      zip((oTa, oTb), rhs_ab)):
                                  nc.tensor.matmul(
                                      oT[:],
                                      lhsT=v1m[m][:, 2 * j + hh, 0:HD + 1],
                                      rhs=rhs,
                                      start=(m == 0), stop=(m == MT - 1))
                          for hh, oT in enumerate((oTa, oTb)):
                              den = rcp.tile([1, 512], F32, tag="den",
                                             name="den")
                              nc.vector.tensor_copy(den[:], oT[HD:HD + 1, :])
                              rc = rcp.tile([1, 512], F32, tag="rc", name="rc")
                              nc.vector.reciprocal_approx_fast(
                                  out=rc[:], in_=den[:])
                              if dump and j == 0 and q5 == 0:
                                  dot = rcp.tile([HD + 1, 512], F32,
                                                 tag="dot", name="dot")
                                  nc.vector.tensor_copy(dot[:], oT[:])
                                  nc.sync.dma_start(
                                      out=ot_dump[:, hh * 512:(hh + 1) * 512],
                                      in_=dot[:])
                                  nc.sync.dma_start(
                                      out=rc_dump[:, hh * 512:(hh + 1) * 512],
                                      in_=rc[:])
                              bc = rbcp.tile([HD, 512], F32, tag="bc",
                                             name="bc")
                              nc.gpsimd.partition_broadcast(bc[:], rc[:])
                              nc.vector.tensor_mul(
                                  oT_sb[j][bass.ts(hh, 64), qsl],
                                  oT[0:HD, :], bc[:])

              if dump:
                  for j in range(2):
                      nc.sync.dma_start(out=qT_dump[j][:], in_=qT[j][:])
                      nc.sync.dma_start(out=kT_dump[j][:], in_=kT[j][:])
                      nc.sync.dma_start(out=o_dump[j][:], in_=oT_sb[j][:])
                  nc.sync.dma_start(
                      out=v_dump[:],
                      in_=v1m[0][:].rearrange("p h d -> p (h d)"))

              # ---- phase 3: projection --------------------------------------
              if not do_proj:
                  for j in range(2):
                      nc.scalar.dma_start(
                          out=yT_d[bass.ts(j, 128), :].bitcast(BF16)[:, 0:N],
                          in_=oT_sb[j][:])
                  return
              with tc.tile_pool(name="yps", bufs=4, space="PSUM") as yps:
                  for jj in range(4):
                      for tch in range(4):
                          yp = yps.tile([128, 512], F32, tag="yp", name="yp")
                          for j in range(2):
                              nc.tensor.matmul(
                                  yp[:],
                                  lhsT=wp_t[j][:, bass.ts(jj, 128)],
                                  rhs=oT_sb[j][:, bass.ts(tch, 512)],
                                  start=(j == 0), stop=(j == 1))
                          ys = ysbp.tile([128, 512], F32, tag="ys", name="ys")
                          if jj % 2 == 0:
                              nc.scalar.copy(ys[:], yp[:])
                          else:
                              nc.vector.tensor_copy(ys[:], yp[:])
                          nc.scalar.dma_start(
                              out=yT_d[bass.ts(jj, 128), bass.ts(tch, 512)],
                              in_=ys[:])

      if reps > 1:
          with tc.For_i(0, reps, 1):
              body()
      else:
          body()

    nc.compile()
    return nc


def get_nc():
    global _NC
    if _NC is None:
        _NC = _build()
    return _NC


def build_timing_nc(reps):
    return _build(reps=reps)


def shard_inputs(x, w_qkv, b_qkv, w_proj, b_proj):
    import ml_dtypes

    bf16 = ml_dtypes.bfloat16
    x = np.asarray(x, dtype=np.float32)
    w_qkv = np.asarray(w_qkv, dtype=np.float32)
    b_qkv = np.asarray(b_qkv, dtype=np.float32)
    w_proj = np.asarray(w_proj, dtype=np.float32)
    ones4 = np.ones((128, HPC), bf16)
    ones_row = np.ones((1, 128), bf16)
    in_maps = []
    for core in range(NCORES):
        b, g = core // 2, core % 2
        sl = slice(g * CS, (g + 1) * CS)
        in_maps.append({
            "xT": np.ascontiguousarray(x[b].T).astype(bf16),
            "wqT": np.ascontiguousarray(w_qkv[sl, :].T).astype(bf16),
            "wkT": np.ascontiguousarray(w_qkv[C:][sl, :].T).astype(bf16),
            "wvT": np.ascontiguousarray(w_qkv[2 * C:][sl, :].T).astype(bf16),
            "wpT": np.ascontiguousarray(w_proj[:, sl].T).astype(bf16),
            "bq": np.ascontiguousarray(b_qkv[sl].reshape(2, 128).T),
            "bk": np.ascontiguousarray(b_qkv[C:][sl].reshape(2, 128).T),
            "bv": np.ascontiguousarray(
                b_qkv[2 * C:][sl].reshape(1, CS)).astype(bf16),
            "ones4": ones4,
            "ones_row": ones_row,
        })
    return in_maps


def gather_output(results, b_proj):
    b_proj = np.asarray(b_proj, dtype=np.float32)
    out = np.empty((B, N, C), np.float32)
    for b in range(B):
        yT = results[2 * b]["yT"] + results[2 * b + 1]["yT"]
        out[b] = yT.T + b_proj[None, :]
    return out


def kernel(x, w_qkv, b_qkv, w_proj, b_proj):
    nc = get_nc()
    in_maps = shard_inputs(x, w_qkv, b_qkv, w_proj, b_proj)
    res = run_bass_kernel_spmd(nc, in_maps, core_ids=list(range(NCORES)))
    return gather_output(res.results, b_proj)
